# revision 11
# baseline (speedup 1.0000x reference)
"""Trainium2 Bass kernel for DeformingPlateModel (MeshGraphNet-style GNN).

Strategy (8 NeuronCores, SPMD):
- Nodes sharded 2500/core (padded to 2560); every edge owned by the core that
  owns its dst. Edges sorted by dst, packed into fixed 128-edge tiles grouped
  by 128-node dst windows (mesh: 6 tiles/group, world: 2, 20 groups/core).
  The group structure is static and identical across cores; per-core variation
  lives in the data (one-hot scatter matrices, gather indices, padding).
- Per step: node latents are AllGathered (bf16); src-side per-edge latents are
  fetched with a transposing dma_gather (feature-major, matmul-ready);
  dst-side latents gather from the core-local AG input buffer, overlapping the
  collective.
- segment_sum = one-hot matmuls accumulated in PSUM (deterministic).
- LayerNorm mean-centering is linear and folded into last-layer weights
  host-side (the model's LN gain=1/bias=0), so LN+residual reduces to
  Square+accum (ACT) -> sqrt -> reciprocal -> one fused scalar_tensor_tensor.
- Hidden activations feature-major (weights-stationary matmuls); the last
  layer of each MLP runs activation-stationary, yielding token-major output so
  the LN tail and residual are native free-axis ops.
"""

import os
import sys

sys.path.insert(0, "/opt/trn_rl_repo")

import numpy as np
import ml_dtypes

from concourse import bacc, tile, mybir
from concourse import bass_utils

BF16 = ml_dtypes.bfloat16
F32 = mybir.dt.float32
BF = mybir.dt.bfloat16
I16 = mybir.dt.int16

NCORES = 8
N = 20000
NPC = 2500          # nodes per core
NPCP = 2560         # padded (20 tiles of 128)
VT = NPCP // 128
LAT = 128
MP_STEPS = int(os.environ.get("KSTEPS", "15"))
LN_EPS = 1e-5

NG = 20             # 128-node groups per core
MTPG, WTPG = 6, 2   # edge tiles per group
MT, WT = NG * MTPG, NG * WTPG
ME, WE = MT * 128, WT * 128
MCT, WCT = ME // 512, WE // 512
NCT = NPCP // 512

IN_DIM, MESH_EF, WORLD_EF = 12, 8, 4

NBLK_STEP = 15
NBLK = 15 * NBLK_STEP + 6 + 3
NBC = 15 * 6 + 6 + 3
NBR = 15 * 3 + 3

RELU = mybir.ActivationFunctionType.Relu
COPY = mybir.ActivationFunctionType.Copy
IDENT = mybir.ActivationFunctionType.Identity
SQUARE = mybir.ActivationFunctionType.Square
SQRT = mybir.ActivationFunctionType.Sqrt
MULT = mybir.AluOpType.mult
ADD = mybir.AluOpType.add

_CACHE = {}


# ----------------------------------------------------------------------------
# host-side preprocessing
# ----------------------------------------------------------------------------

def _np(x):
    return np.asarray(x)


def _fold_mlp(p, ln):
    (w1, b1), (w2, b2), (w3, b3) = [
        (_np(w).astype(np.float64), _np(b).astype(np.float64)) for w, b in p["layers"]]
    if ln:
        g, bt = [_np(a) for a in p["ln"]]
        assert np.allclose(g, 1.0) and np.allclose(bt, 0.0), "nontrivial LN affine"
        o = w3.shape[1]
        c = np.eye(o) - 1.0 / o
        w3, b3 = w3 @ c, b3 @ c
    return [(w1, b1), (w2, b2), (w3, b3)]


def _fold_invnorm(layers, stats):
    mean, std = [_np(a).astype(np.float64) for a in stats]
    w1, b1 = layers[0]
    layers[0] = (w1 / std[:, None], b1 - (mean / std) @ w1)
    return layers


def _pack_edges(srcs, dsts, core, tpg):
    srcs = srcs.astype(np.int64)
    dsts = dsts.astype(np.int64)
    sel = np.nonzero(dsts // NPC == core)[0]
    ld = dsts[sel] - NPC * core
    order = np.argsort(ld, kind="stable")
    sel, ld = sel[order], ld[order]
    cap = tpg * 128
    perm = np.zeros(NG * cap, np.int64)
    valid = np.zeros(NG * cap, bool)
    ldo = np.zeros(NG * cap, np.int64)
    for g in range(NG):
        m = (ld // 128) == g
        k = int(m.sum())
        assert k <= cap, f"group overflow: {k} > {cap}"
        perm[g * cap:g * cap + k] = sel[m]
        valid[g * cap:g * cap + k] = True
        ldo[g * cap:g * cap + k] = ld[m]
        ldo[g * cap + k:(g + 1) * cap] = g * 128
    return perm, valid, ldo


def _wrap_idx(idx, cap):
    a = np.asarray(idx, np.int16).reshape(cap // 16, 16).T
    return np.tile(a, (8, 1)).copy()


def _onehot_smat(ldo, valid, tpg):
    nt = NG * tpg
    s = np.zeros((nt, 128, 128), np.float32)
    t_idx = np.arange(nt * 128) // 128
    g_idx = t_idx // tpg
    rows = np.arange(nt * 128) % 128
    cols = ldo - g_idx * 128
    vv = valid
    s[t_idx[vv], rows[vv], cols[vv]] = 1.0
    return s.astype(BF16)


def _prep(node_type, mesh_pos, world_pos, known_vel, srcs, dsts, wsrcs, wdsts,
          params):
    node_type = _np(node_type)
    mesh_pos = _np(mesh_pos).astype(np.float64)
    world_pos = _np(world_pos).astype(np.float64)
    known_vel = _np(known_vel).astype(np.float64)
    srcs, dsts = _np(srcs), _np(dsts)
    wsrcs, wdsts = _np(wsrcs), _np(wdsts)

    oh = np.zeros((N, 9))
    oh[np.arange(N), node_type.astype(np.int64)] = 1.0
    node_f = np.concatenate([known_vel, oh], -1)
    rm = mesh_pos[srcs] - mesh_pos[dsts]
    rwm = world_pos[srcs] - mesh_pos[dsts]
    mesh_ef = np.concatenate([rm, np.linalg.norm(rm, axis=-1, keepdims=True),
                              rwm, np.linalg.norm(rwm, axis=-1, keepdims=True)], -1)
    rw = world_pos[wsrcs] - world_pos[wdsts]
    world_ef = np.concatenate([rw, np.linalg.norm(rw, axis=-1, keepdims=True)], -1)

    nrm = params["norms"]
    enc_node = _fold_invnorm(_fold_mlp(params["node_enc"], True), nrm["node"])
    enc_mesh = _fold_invnorm(_fold_mlp(params["mesh_enc"], True), nrm["mesh"])
    enc_world = _fold_invnorm(_fold_mlp(params["world_enc"], True), nrm["world"])
    blocks = [{"mesh": _fold_mlp(b["edge"][0], True),
               "world": _fold_mlp(b["edge"][1], True),
               "node": _fold_mlp(b["node"], True)} for b in params["blocks"]]
    dec = _fold_mlp(params["dec"], False)
    mo, so = [_np(a).astype(np.float64) for a in nrm["out"]]
    dec[2] = (dec[2][0] * so[None, :], dec[2][1] * so + mo)

    wblocks = np.zeros((NBLK, 128, 128), np.float32)
    biascol = np.zeros((128, NBC), np.float32)
    biasrow = np.zeros((NBR, 512), np.float32)
    for s, blk in enumerate(blocks):
        for mi, key in enumerate(("mesh", "world", "node")):
            (w1, b1), (w2, b2), (w3, b3) = blk[key]
            base = s * NBLK_STEP + mi * 5
            wblocks[base + 0] = w1[0:128]
            wblocks[base + 1] = w1[128:256]
            wblocks[base + 2] = w1[256:384]
            wblocks[base + 3] = w2
            wblocks[base + 4] = w3
            biascol[:, s * 6 + mi * 2 + 0] = b1
            biascol[:, s * 6 + mi * 2 + 1] = b2
            biasrow[s * 3 + mi] = np.tile(b3, 4)
    eb = 15 * NBLK_STEP
    for i, enc in enumerate((enc_node, enc_mesh, enc_world)):
        wblocks[eb + 2 * i + 0] = enc[1][0]
        wblocks[eb + 2 * i + 1] = enc[2][0]
        biascol[:, 90 + 2 * i + 0] = enc[0][1]
        biascol[:, 90 + 2 * i + 1] = enc[1][1]
        biasrow[45 + i] = np.tile(enc[2][1], 4)
    db = eb + 6
    wblocks[db + 0] = dec[0][0]
    wblocks[db + 1] = dec[1][0]
    wblocks[db + 2, :, :3] = dec[2][0]
    biascol[:, 96] = dec[0][1]
    biascol[:, 97] = dec[1][1]
    biascol[:3, 98] = dec[2][1]

    shared = {
        "wmain": wblocks.astype(BF16),
        "wenc_node": enc_node[0][0].astype(BF16),
        "wenc_mesh": enc_mesh[0][0].astype(BF16),
        "wenc_world": enc_world[0][0].astype(BF16),
        "biascol": biascol,
        "biasrow": biasrow,
        "identf": np.eye(128, dtype=np.float32),
    }

    in_maps = []
    for c in range(NCORES):
        mperm, mvalid, mld = _pack_edges(srcs, dsts, c, MTPG)
        wperm, wvalid, wld = _pack_edges(wsrcs, wdsts, c, WTPG)
        pad_gid = lambda n: NPCP * (n // NPC) + n % NPC
        msrc_g = np.where(mvalid, srcs[mperm], 0)
        wsrc_g = np.where(wvalid, wsrcs[wperm], 0)
        mef = np.where(mvalid[:, None], mesh_ef[mperm], 0.0)
        wef = np.where(wvalid[:, None], world_ef[wperm], 0.0)
        m = {
            "nodef": np.pad(node_f[c * NPC:(c + 1) * NPC],
                            ((0, NPCP - NPC), (0, 0))).T.astype(BF16).copy(),
            "meshef": mef.T.astype(BF16).copy(),
            "worldef": wef.T.astype(BF16).copy(),
            "msrc": _wrap_idx(pad_gid(msrc_g), ME),
            "mdst": _wrap_idx(mld, ME),
            "wsrc": _wrap_idx(pad_gid(wsrc_g), WE),
            "wdst": _wrap_idx(wld, WE),
            "smat_m": _onehot_smat(mld, mvalid, MTPG),
            "smat_w": _onehot_smat(wld, wvalid, WTPG),
        }
        m.update(shared)
        in_maps.append(m)
    return in_maps


# ----------------------------------------------------------------------------
# kernel builder
# ----------------------------------------------------------------------------

def build_nc(mp_steps=MP_STEPS):
    nc = bacc.Bacc(None, target_bir_lowering=False)
    d = {}
    d["nodef"] = nc.dram_tensor("nodef", [IN_DIM, NPCP], BF, kind="ExternalInput")
    d["meshef"] = nc.dram_tensor("meshef", [MESH_EF, ME], BF, kind="ExternalInput")
    d["worldef"] = nc.dram_tensor("worldef", [WORLD_EF, WE], BF, kind="ExternalInput")
    for nm, cap in (("msrc", ME), ("mdst", ME), ("wsrc", WE), ("wdst", WE)):
        d[nm] = nc.dram_tensor(nm, [128, cap // 16], I16, kind="ExternalInput")
    d["smat_m"] = nc.dram_tensor("smat_m", [MT, 128, 128], BF, kind="ExternalInput")
    d["smat_w"] = nc.dram_tensor("smat_w", [WT, 128, 128], BF, kind="ExternalInput")
    d["wmain"] = nc.dram_tensor("wmain", [NBLK, 128, 128], BF, kind="ExternalInput")
    d["wenc_node"] = nc.dram_tensor("wenc_node", [IN_DIM, 128], BF, kind="ExternalInput")
    d["wenc_mesh"] = nc.dram_tensor("wenc_mesh", [MESH_EF, 128], BF, kind="ExternalInput")
    d["wenc_world"] = nc.dram_tensor("wenc_world", [WORLD_EF, 128], BF,
                                     kind="ExternalInput")
    d["biascol"] = nc.dram_tensor("biascol", [128, NBC], F32, kind="ExternalInput")
    d["biasrow"] = nc.dram_tensor("biasrow", [NBR, 512], F32, kind="ExternalInput")
    d["identf"] = nc.dram_tensor("identf", [128, 128], F32, kind="ExternalInput")
    out = nc.dram_tensor("out", [3, NPCP], F32, kind="ExternalOutput")
    dbg = None
    if os.environ.get("KDEBUG", "0") == "1":
        dbg = {
            "dbg_v": nc.dram_tensor("dbg_v", [NPCP, 128], F32, kind="ExternalOutput"),
            "dbg_em": nc.dram_tensor("dbg_em", [ME, 128], F32, kind="ExternalOutput"),
            "dbg_aggm": nc.dram_tensor("dbg_aggm", [128, NPCP], F32,
                                       kind="ExternalOutput"),
        }
    d["_dbg"] = dbg

    ag_ins = [nc.dram_tensor(f"ag_in{s}", [NPCP, LAT], BF, kind="Internal")
              for s in range(mp_steps)]
    ag_outs = [nc.dram_tensor(f"ag_out{s}", [NPCP * NCORES, LAT], BF,
                              kind="Internal", addr_space="Shared")
               for s in range(mp_steps)]

    with tile.TileContext(nc) as tc:
        _build_body(nc, tc, d, out, ag_ins, ag_outs, mp_steps)
    nc.compile()
    return nc


def _build_body(nc, tc, d, out, ag_ins, ag_outs, mp_steps):
    from contextlib import ExitStack
    es = ExitStack()
    const = es.enter_context(tc.tile_pool(name="const", bufs=1))
    state = es.enter_context(tc.tile_pool(name="state", bufs=1))
    wpool = es.enter_context(tc.tile_pool(name="wpool", bufs=2))
    brow = es.enter_context(tc.tile_pool(name="brow", bufs=4))
    gath = es.enter_context(tc.tile_pool(name="gath", bufs=2))
    work2 = es.enter_context(tc.tile_pool(name="work2", bufs=2))
    work1 = es.enter_context(tc.tile_pool(name="work1", bufs=1))
    small = es.enter_context(tc.tile_pool(name="small", bufs=4))
    aggp = es.enter_context(tc.tile_pool(name="aggp", bufs=1))
    vfmp = es.enter_context(tc.tile_pool(name="vfmp", bufs=1))
    ps_l1 = es.enter_context(tc.tile_pool(name="ps_l1", bufs=3, space="PSUM"))
    ps_tm = es.enter_context(tc.tile_pool(name="ps_tm", bufs=3, space="PSUM"))
    ps_agg = es.enter_context(tc.tile_pool(name="ps_agg", bufs=2, space="PSUM"))

    # ---- resident constants
    idx_m_src = const.tile([128, ME // 16], I16, tag="ims")
    idx_m_dst = const.tile([128, ME // 16], I16, tag="imd")
    idx_w_src = const.tile([128, WE // 16], I16, tag="iws")
    idx_w_dst = const.tile([128, WE // 16], I16, tag="iwd")
    nc.sync.dma_start(idx_m_src[:], d["msrc"][:, :])
    nc.sync.dma_start(idx_m_dst[:], d["mdst"][:, :])
    nc.sync.dma_start(idx_w_src[:], d["wsrc"][:, :])
    nc.sync.dma_start(idx_w_dst[:], d["wdst"][:, :])
    bcol = const.tile([128, NBC], F32, tag="bcol")
    nc.sync.dma_start(bcol[:], d["biascol"][:, :])
    identf = const.tile([128, 128], F32, tag="identf")
    nc.sync.dma_start(identf[:], d["identf"][:, :])
    identb = const.tile([128, 128], BF, tag="identb")
    nc.vector.tensor_copy(identb[:], identf[:])
    smat_m = const.tile([128, MT, 128], BF, tag="smm")
    nc.sync.dma_start(smat_m[:], d["smat_m"].rearrange("t p f -> p t f"))
    smat_w = const.tile([128, WT, 128], BF, tag="smw")
    nc.sync.dma_start(smat_w[:], d["smat_w"].rearrange("t p f -> p t f"))
    eps_t = const.tile([128, 1], F32, tag="eps")
    nc.vector.memset(eps_t[:], LN_EPS)

    # ---- state
    v_tm = state.tile([128, VT, 128], F32, tag="v")
    e_m = state.tile([128, MT, 128], BF, tag="em")
    e_w = state.tile([128, WT, 128], BF, tag="ew")

    def load_bias_row(row):
        r = small.tile([1, 512], F32, tag="brow_src")
        nc.sync.dma_start(r[:], d["biasrow"][row:row + 1, :])
        b = brow.tile([128, 512], F32, tag="brow")
        nc.gpsimd.partition_broadcast(b[:], r[:])
        return b

    def load_wblocks(idx_list):
        t = wpool.tile([128, NBLK_STEP, 128], BF, tag="wstep")
        for i, bi in enumerate(idx_list):
            nc.sync.dma_start(t[:, i, :], d["wmain"][bi, :, :])
        return t

    def ln_tail(ps3, b3rep, state3, residual):
        """state = (ps3 + b3) / sigma [+ state].
        ps3/state3: [128, 4, 128] APs; LN stats are per (partition, sub-tile)."""
        t = work2.tile([128, 4, 128], F32, tag="tmt")
        nc.vector.tensor_add(t[:, :, :].rearrange("p a b -> p (a b)"),
                             ps3[:, :, :].rearrange("p a b -> p (a b)"),
                             b3rep[:, 0:512])
        junk = work1.tile([128, 128], F32, tag="junk")
        for m in range(4):
            ss = small.tile([128, 1], F32, tag="ss")
            nc.scalar.activation(junk[:, :], t[:, m, :], SQUARE,
                                 accum_out=ss[:, :])
            sig = small.tile([128, 1], F32, tag="sig")
            nc.scalar.activation(sig[:], ss[:], SQRT, scale=1.0 / 128.0,
                                 bias=eps_t[:, :])
            rinv = small.tile([128, 1], F32, tag="rinv")
            nc.vector.reciprocal(rinv[:], sig[:])
            if residual:
                nc.vector.scalar_tensor_tensor(state3[:, m, :], t[:, m, :],
                                               rinv[:, :], state3[:, m, :],
                                               op0=MULT, op1=ADD)
            else:
                nc.vector.tensor_scalar_mul(state3[:, m, :], t[:, m, :],
                                            rinv[:, :])

    def mlp_core(T, l1_parts, b1col, w2, b2col, w3, b3rep, tail_cb):
        for cj in range(T // 512):
            ps = ps_l1.tile([128, 512], F32, tag="psl1")
            npp = len(l1_parts)
            for i, (w, rf) in enumerate(l1_parts):
                nc.tensor.matmul(ps[:, :], w, rf(cj), start=(i == 0),
                                 stop=(i == npp - 1))
            h1 = work2.tile([128, 512], BF, tag="h1")
            nc.scalar.activation(h1[:], ps[:], RELU, bias=b1col)
            ps2 = ps_l1.tile([128, 512], F32, tag="psl1")
            nc.tensor.matmul(ps2[:, :], w2, h1[:, :], start=True, stop=True)
            h2 = work2.tile([128, 512], BF, tag="h2")
            nc.scalar.activation(h2[:], ps2[:], RELU, bias=b2col)
            ps3 = ps_tm.tile([128, 4, 128], F32, tag="pstm")
            for m in range(4):
                nc.tensor.matmul(ps3[:, m, :], h2[:, m * 128:(m + 1) * 128],
                                 w3, start=True, stop=True)
            tail_cb(cj, ps3)

    def transpose_in(src_tm, t0, n, src_f32):
        """PE-transpose n [128,128] TM tiles -> one FM SBUF bf16 tile [128,n*128]."""
        pdt = F32 if src_f32 else BF
        idn = identf if src_f32 else identb
        pst = ps_tm.tile([128, n, 128], pdt, tag="pstm")
        for i in range(n):
            nc.tensor.transpose(pst[:, i, :], src_tm[:, t0 + i, :], idn[:, :])
        o = work2.tile([128, n * 128], BF, tag="efm")
        nc.scalar.activation(o[:, :], pst[:, :, :].rearrange("p a b -> p (a b)"),
                             COPY)
        return o

    def make_scatter(e_state, smat, tpg, agg_sbuf):
        ctx = {"pt": None, "cid": -1}

        def scatter(t0):
            for t in range(t0, t0 + 4):
                g, k = divmod(t, tpg)
                c, gc = divmod(g, 4)
                if ctx["cid"] != c:
                    ctx["pt"] = ps_agg.tile([128, 512], F32, tag="psagg",
                                            name="psagg_t")
                    ctx["cid"] = c
                pt = ctx["pt"]
                nc.tensor.matmul(pt[:, gc * 128:(gc + 1) * 128],
                                 e_state[:, t, :], smat[:, t, :],
                                 start=(k == 0), stop=(k == tpg - 1))
                if gc == 3 and k == tpg - 1:
                    nc.scalar.activation(agg_sbuf[:, c * 512:(c + 1) * 512],
                                         pt[:, :], COPY)
        return scatter

    def edge_tail_factory(e_state, base_ct, b3rep, scatter):
        def cb(cj, ps3):
            t0 = 4 * (base_ct + cj)
            ln_tail(ps3, b3rep, e_state[:, t0:t0 + 4, :], residual=True)
            scatter(t0)
        return cb

    def edge_set(e_state, smat, tpg, idx_src, idx_dst, ag_in, ag_out,
                 wt, w_off, b1c, b2c, b3rep, agg_sbuf, gtag):
        ecap = NG * tpg * 128
        half = ecap // 2
        hct = half // 512
        scat = make_scatter(e_state, smat, tpg, agg_sbuf)
        for h in range(2):
            vd = gath.tile([128, 1, half], BF, tag=gtag)
            nc.gpsimd.dma_gather(
                out_ap=vd[:, :, :], in_ap=ag_in[:, :],
                idxs_ap=idx_dst[:, h * (half // 16):(h + 1) * (half // 16)],
                num_idxs=half, num_idxs_reg=half, elem_size=LAT, transpose=True,
                single_packet=False)
            vs = gath.tile([128, 1, half], BF, tag=gtag)
            nc.gpsimd.dma_gather(
                out_ap=vs[:, :, :], in_ap=ag_out[:, :],
                idxs_ap=idx_src[:, h * (half // 16):(h + 1) * (half // 16)],
                num_idxs=half, num_idxs_reg=half, elem_size=LAT, transpose=True,
                single_packet=False)

            def efm_rhs(cj, _b=h * (half // 128)):
                return transpose_in(e_state, _b + 4 * cj, 4, False)[:, :]

            parts = [
                (wt[:, w_off + 0, :], lambda cj, v=vs: v[:, 0, cj * 512:(cj + 1) * 512]),
                (wt[:, w_off + 1, :], lambda cj, v=vd: v[:, 0, cj * 512:(cj + 1) * 512]),
                (wt[:, w_off + 2, :], efm_rhs),
            ]
            mlp_core(half, parts, b1c, wt[:, w_off + 3, :], b2c,
                     wt[:, w_off + 4, :], b3rep,
                     edge_tail_factory(e_state, h * hct, b3rep, scat))

    # ------------------------------------------------------------------
    # encoders
    # ------------------------------------------------------------------
    eb = 15 * NBLK_STEP
    db = eb + 6
    enc_w = load_wblocks([eb + i for i in range(6)])

    def encode(dram_f, kdim, wenc_dram, total, b3row, state_t, wi, bc, gtag):
        """Encoder MLP over `total` tokens, feature chunks via the gath pool."""
        wen = small.tile([kdim, 128], BF, tag="wen" + gtag, name="wen_t")
        nc.sync.dma_start(wen[:], wenc_dram[:, :])
        b3 = load_bias_row(b3row)
        chunk = min(total, ME // 2)
        for h in range(0, total, chunk):
            f = gath.tile([kdim, chunk], BF, tag=gtag, name="encf_t")
            nc.sync.dma_start(f[:], dram_f[:, h:h + chunk])
            base = h // 512
            mlp_core(chunk, [(wen[:, :],
                              lambda cj, ff=f: ff[:, cj * 512:(cj + 1) * 512])],
                     bcol[:, bc:bc + 1], enc_w[:, wi, :], bcol[:, bc + 1:bc + 2],
                     enc_w[:, wi + 1, :], b3,
                     lambda cj, ps3, _b=base: ln_tail(
                         ps3, b3, state_t[:, 4 * (_b + cj):4 * (_b + cj) + 4, :],
                         residual=False))

    encode(d["nodef"], IN_DIM, d["wenc_node"], NPCP, 45, v_tm, 0, 90, "gm")
    encode(d["meshef"], MESH_EF, d["wenc_mesh"], ME, 46, e_m, 2, 92, "gm")
    encode(d["worldef"], WORLD_EF, d["wenc_world"], WE, 47, e_w, 4, 94, "gw")

    # ------------------------------------------------------------------
    # message-passing steps
    # ------------------------------------------------------------------
    for s in range(mp_steps):
        ag_in, ag_out = ag_ins[s], ag_outs[s]
        nc.gpsimd.dma_start(ag_in.rearrange("(t p) f -> p t f", p=128), v_tm[:])
        nc.gpsimd.collective_compute(
            "AllGather", mybir.AluOpType.bypass,
            replica_groups=[list(range(NCORES))],
            ins=[ag_in[:, :]], outs=[ag_out[:, :]])

        wt = load_wblocks(list(range(s * NBLK_STEP, (s + 1) * NBLK_STEP)))
        b3_mesh = load_bias_row(s * 3 + 0)
        b3_world = load_bias_row(s * 3 + 1)
        b3_node = load_bias_row(s * 3 + 2)
        aggm = aggp.tile([128, NPCP], BF, tag="aggm")
        aggw = aggp.tile([128, NPCP], BF, tag="aggw")

        edge_set(e_m, smat_m, MTPG, idx_m_src, idx_m_dst, ag_in, ag_out,
                 wt, 0, bcol[:, s * 6 + 0:s * 6 + 1], bcol[:, s * 6 + 1:s * 6 + 2],
                 b3_mesh, aggm, "gm")
        edge_set(e_w, smat_w, WTPG, idx_w_src, idx_w_dst, ag_in, ag_out,
                 wt, 5, bcol[:, s * 6 + 2:s * 6 + 3], bcol[:, s * 6 + 3:s * 6 + 4],
                 b3_world, aggw, "gw")

        vfm = vfmp.tile([128, NPCP], BF, tag="vfm")
        for cj in range(NCT):
            o = transpose_in(v_tm, 4 * cj, 4, True)
            nc.vector.tensor_copy(vfm[:, cj * 512:(cj + 1) * 512], o[:, :])

        def node_tail(cj, ps3, b3=b3_node):
            ln_tail(ps3, b3, v_tm[:, 4 * cj:4 * cj + 4, :], residual=True)

        mlp_core(NPCP, [
            (wt[:, 10, :], lambda cj, v=vfm: v[:, cj * 512:(cj + 1) * 512]),
            (wt[:, 11, :], lambda cj, a=aggm: a[:, cj * 512:(cj + 1) * 512]),
            (wt[:, 12, :], lambda cj, a=aggw: a[:, cj * 512:(cj + 1) * 512]),
        ], bcol[:, s * 6 + 4:s * 6 + 5], wt[:, 13, :],
            bcol[:, s * 6 + 5:s * 6 + 6], wt[:, 14, :], b3_node, node_tail)

    # ------------------------------------------------------------------
    # decoder
    # ------------------------------------------------------------------
    dec_w = load_wblocks([db + 0, db + 1, db + 2])
    outt = work1.tile([3, NPCP], F32, tag="outt")
    vfm = vfmp.tile([128, NPCP], BF, tag="vfm")
    for cj in range(NCT):
        o = transpose_in(v_tm, 4 * cj, 4, True)
        nc.vector.tensor_copy(vfm[:, cj * 512:(cj + 1) * 512], o[:, :])
    for cj in range(NCT):
        ps = ps_l1.tile([128, 512], F32, tag="psl1")
        nc.tensor.matmul(ps[:, :], dec_w[:, 0, :],
                         vfm[:, cj * 512:(cj + 1) * 512], start=True, stop=True)
        h1 = work2.tile([128, 512], BF, tag="h1")
        nc.scalar.activation(h1[:], ps[:], RELU, bias=bcol[:, 96:97])
        ps2 = ps_l1.tile([128, 512], F32, tag="psl1")
        nc.tensor.matmul(ps2[:, :], dec_w[:, 1, :], h1[:, :], start=True, stop=True)
        h2 = work2.tile([128, 512], BF, tag="h2")
        nc.scalar.activation(h2[:], ps2[:], RELU, bias=bcol[:, 97:98])
        ps3 = ps_l1.tile([128, 512], F32, tag="psl1")
        nc.tensor.matmul(ps3[:, :], dec_w[:, 2, :], h2[:, :], start=True, stop=True)
        nc.scalar.activation(outt[:, cj * 512:(cj + 1) * 512], ps3[0:3, :],
                             IDENT, bias=bcol[0:3, 98:99])
    nc.sync.dma_start(out[:, :], outt[:, :])
    if d["_dbg"] is not None:
        nc.gpsimd.dma_start(
            d["_dbg"]["dbg_v"].rearrange("(t p) f -> p t f", p=128), v_tm[:])
        nc.gpsimd.dma_start(
            d["_dbg"]["dbg_em"].rearrange("(t p) f -> p t f", p=128), e_m[:])
    es.close()


# ----------------------------------------------------------------------------
# entry point
# ----------------------------------------------------------------------------

def kernel(node_type, mesh_pos, world_pos, known_vel, srcs, dsts, wsrcs, wdsts,
           params, _trace=False):
    in_maps = _prep(node_type, mesh_pos, world_pos, known_vel, srcs, dsts,
                    wsrcs, wdsts, params)
    if "nc" not in _CACHE:
        _CACHE["nc"] = build_nc()
    nc = _CACHE["nc"]
    res = bass_utils.run_bass_kernel_spmd(nc, in_maps,
                                          core_ids=list(range(NCORES)),
                                          trace=_trace)
    outs = [res.results[c]["out"] for c in range(NCORES)]
    full = np.concatenate([o[:, :NPC].T for o in outs], 0).astype(np.float32)
    if _trace:
        return full, res
    return full
